# revision 25
# baseline (speedup 1.0000x reference)
# BitLinear 1.58 (ternary-weight linear with int8 activation quantization)
# on 8 Trainium2 NeuronCores via Bass/Tile.
#
# Reference computation (fp32):
#   w_scale = max(mean(|W|), 1e-5)           (global over the full weight)
#   W_q     = clip(round(W / w_scale), -1, 1)          (ternary)
#   gamma   = max(max(|x|), 1e-5)            (global over the full activation)
#   x_q     = clip(round(x * 128/gamma), -128, 127)
#   out     = (x_q @ W_q^T) * (gamma*w_scale/128) + bias
#
# Sharding: data-parallel over the 8192 tokens (1024 tokens/core), weight
# replicated. Global scales via per-core partial stats + tiny AllGathers
# (w-stats gathered first so the W pipeline warms while x-stats finish).
#
# Matmul: W_q is the STATIONARY operand (of-chunks of 128 on PSUM
# partitions), x_q streams (512-token halves), output transposed
# ([OUT_F, TPC] per core, host transposes back). Of the 32 k-tiles:
#   - k < FK: fp8-e4m3 DoubleRow, two k-tiles per matmul (2 MACs/cell/cyc).
#     x_q rounds to e4m3; measured+simulated rel_err 0.0157 at FK=20 vs the
#     2e-2 gate (error scales ~sqrt(FK), 0.0196 at FK=32).
#   - k >= FK: exact bf16 matmuls (x_q in [-128,127] and W_q in {-1,0,1}
#     are exact in bf16; PSUM accumulates fp32, sums < 2^24).
# Mixing fp8-DoubleRow and bf16 matmuls in one PSUM accumulation group is
# fine - accumulation is fp32 either way.
#
# x is read once for stats; the FK paired k-tiles stay resident in SBUF
# (80 KB/part) so they quantize the moment gamma lands. The 12 exact
# k-tiles are re-read on the scalar ring (idle after the 8MB wS scan) and
# group 0's matmul emission is split resident-first so the PE has work for
# all 4 chunks while the re-read streams.
#
# Rounding: round-half-to-even (= jnp.round) done exactly in fp32 via the
# magic-constant trick (v + 1.5*2^23) - 1.5*2^23, fused into tensor_scalar.
# The int->e4m3 conversion on DVE output is RNE, matching the error sim.

import numpy as np
from contextlib import ExitStack

import concourse.bass as bass
import concourse.tile as tile
from concourse import bacc, mybir
from concourse import bass_utils

N_CORES = 8
IN_F = 4096
OUT_F = 4096
TOKENS = 8192  # 4 * 2048
TPC = TOKENS // N_CORES  # tokens per core = 1024
OSL = OUT_F // N_CORES  # per-core weight-stats slice = 512 out_features

KT = IN_F // 128  # 32 k-tiles
FK = 20  # k-tiles [0, FK) fp8-paired, [FK, KT) exact bf16
NG = OUT_F // 512  # 8 of-groups of 512
CPG = 4  # chunks (128 of) per group

MAGIC = 12582912.0  # 1.5 * 2**23: (v + MAGIC) - MAGIC == round-half-even(v)
EPS = 1e-5
F32 = mybir.dt.float32
FP8 = mybir.dt.float8e4
BF16 = mybir.dt.bfloat16
DR = mybir.MatmulPerfMode.DoubleRow

_cache = {}


def _build():
    nc = bacc.Bacc("TRN2", target_bir_lowering=False, debug=False, num_devices=N_CORES)
    xT = nc.dram_tensor("xT", [IN_F, TPC], F32, kind="ExternalInput").ap()
    wT = nc.dram_tensor("wT", [IN_F, OUT_F], F32, kind="ExternalInput").ap()
    wS = nc.dram_tensor("wS", [IN_F, OSL], F32, kind="ExternalInput").ap()
    bias = nc.dram_tensor("bias", [OUT_F], F32, kind="ExternalInput").ap()
    outT = nc.dram_tensor("outT", [OUT_F, TPC], F32, kind="ExternalOutput").ap()

    with tile.TileContext(nc) as tc, ExitStack() as ctx:
        ep = ctx.enter_context
        singles = ep(tc.tile_pool(name="singles", bufs=1))
        psum_pool = ep(tc.tile_pool(name="psum", bufs=8, space="PSUM"))
        dram = ep(tc.tile_pool(name="dram", bufs=1, space="DRAM"))
        # main pools first so the stats scope below is innermost (LIFO)
        xq_pool = ep(tc.tile_pool(name="xq", bufs=FK // 2 + (KT - FK)))
        win_pool = ep(tc.tile_pool(name="win", bufs=4))
        wq_pool = ep(tc.tile_pool(name="wq", bufs=2 * (FK // 2 + (KT - FK))))
        xin_pool = ep(tc.tile_pool(name="xin", bufs=3))
        # paired k-tiles of x stay resident through the stats phase
        sctx = ExitStack()
        xres_pool = sctx.enter_context(tc.tile_pool(name="xres", bufs=FK))
        spw_pool = sctx.enter_context(tc.tile_pool(name="spw", bufs=2))
        spx_pool = sctx.enter_context(tc.tile_pool(name="spx", bufs=2))

        ones_row = singles.tile([1, 128], F32)  # for partition-broadcast matmul
        nc.vector.memset(ones_row[:], 1.0)
        # warm-up AllGather: absorbs the collective launch/barrier cost
        # while stats stream
        warm_i = dram.tile([1], F32, tag="warm")
        warm_o = dram.tile([N_CORES], F32, tag="warmo")
        nc.scalar.dma_start(warm_i[:], ones_row[0:1, 0:1])
        nc.gpsimd.collective_compute(
            "AllGather", mybir.AluOpType.bypass,
            replica_groups=[list(range(N_CORES))],
            ins=[warm_i.opt()], outs=[warm_o.opt()],
        )
        # ---- stats: x loads split across two rings (sync+gpsimd); k < FK
        # kept resident. wS streams on the scalar ring with |.| accumulated
        # by ACT so all three rings pull concurrently.
        xm = singles.tile([128, KT], F32)
        xres = []
        for k in range(KT):
            if k < FK:
                xt = xres_pool.tile([128, TPC], F32, tag="xres", name=f"xres{k}")
                xres.append(xt)
            else:
                xt = spx_pool.tile([128, TPC], F32, tag="spx", name=f"spx{k}")
            eng = nc.sync if k % 2 == 0 else nc.gpsimd
            eng.dma_start(xt[:], xT[k * 128 : (k + 1) * 128, :])
            nc.vector.tensor_reduce(
                xm[:, k : k + 1], xt[:], axis=mybir.AxisListType.X,
                op=mybir.AluOpType.max, apply_absolute_value=True,
            )
        SW = 1024
        wrows = SW // OSL
        NWS = IN_F // (128 * wrows)
        wv = wS[:].rearrange("(a p x) y -> a p (x y)", p=128, x=wrows)
        wm = singles.tile([128, NWS], F32)
        for j in range(NWS):
            st = spw_pool.tile([128, SW], F32, tag="spw", name=f"sw{j}")
            nc.scalar.dma_start(st[:], wv[j])
            nc.scalar.activation(
                st[:], st[:], mybir.ActivationFunctionType.Abs,
                accum_out=wm[:, j : j + 1],
            )

        # fold [128,N] -> [1,1] (cross-partition via DMA reshape). The tiny
        # DMAs ride the ring whose big reads gate them anyway (w-side ->
        # scalar behind wS, x-side -> sync behind x-stats), so they never
        # queue behind unrelated megabytes.
        def fold(src, op, nm, eng):
            c = singles.tile([128, 1], F32, tag=f"{nm}c")
            nc.vector.tensor_reduce(c[:], src[:], axis=mybir.AxisListType.X, op=op)
            t = singles.tile([1, 128], F32, tag=f"{nm}t")
            eng.dma_start(t[:], c[:])
            r = singles.tile([1, 1], F32, tag=f"{nm}r")
            nc.vector.tensor_reduce(r[:], t[:], axis=mybir.AxisListType.X, op=op)
            return r

        wsum = fold(wm, mybir.AluOpType.add, "ws", nc.scalar)
        gx = fold(xm, mybir.AluOpType.max, "gx", nc.sync)


        def newton_recip(name, src):
            # correctly-rounded-ish 1/src: HW reciprocal + one Newton step
            r0 = singles.tile([1, 1], F32, tag=f"{name}r0")
            nc.vector.reciprocal(r0[:], src[:])
            t = singles.tile([1, 1], F32, tag=f"{name}t")
            nc.vector.tensor_tensor(t[:], src[:], r0[:], op=mybir.AluOpType.mult)
            u = singles.tile([1, 1], F32, tag=f"{name}u")
            nc.vector.tensor_scalar(
                u[:], t[:], -1.0, 2.0, mybir.AluOpType.mult, mybir.AluOpType.add
            )
            r1 = singles.tile([1, 1], F32, tag=f"{name}r1")
            nc.vector.tensor_tensor(r1[:], r0[:], u[:], op=mybir.AluOpType.mult)
            return r1

        # ---- one combined AllGather for both stats ----
        cc_sb = singles.tile([1, 2], F32)
        nc.vector.tensor_copy(cc_sb[0:1, 0:1], gx[:])
        nc.vector.tensor_copy(cc_sb[0:1, 1:2], wsum[:])
        cin = dram.tile([2], F32, tag="cci")
        cout = dram.tile([2 * N_CORES], F32, tag="cco")
        nc.sync.dma_start(cin[:], cc_sb[:])
        nc.gpsimd.collective_compute(
            "AllGather", mybir.AluOpType.bypass,
            replica_groups=[list(range(N_CORES))],
            ins=[cin.opt()], outs=[cout.opt()],
        )
        g16 = singles.tile([1, 2 * N_CORES], F32)
        nc.sync.dma_start(g16[:], cout[:])
        g3 = g16[:].rearrange("p (r two) -> p two r", two=2)

        gsum = singles.tile([1, 1], F32)
        nc.vector.tensor_reduce(
            gsum[:], g3[0:1, 1:2, :], axis=mybir.AxisListType.X,
            op=mybir.AluOpType.add,
        )
        wscale = singles.tile([1, 1], F32)
        nc.vector.tensor_scalar(
            wscale[:], gsum[:], 1.0 / (OUT_F * IN_F), EPS,
            mybir.AluOpType.mult, mybir.AluOpType.max,
        )
        gmax = singles.tile([1, 1], F32)
        nc.vector.tensor_reduce(
            gmax[:], g3[0:1, 0:1, :], axis=mybir.AxisListType.X,
            op=mybir.AluOpType.max,
        )
        gamma = singles.tile([1, 1], F32)
        nc.vector.tensor_scalar(gamma[:], gmax[:], EPS, None, mybir.AluOpType.max)
        rw = newton_recip("rw", wscale)  # 1/w_scale
        rg = newton_recip("rg", gamma)   # 1/gamma
        pack3 = singles.tile([1, 3], F32)
        nc.vector.tensor_scalar(
            pack3[0:1, 0:1], rg[:], 128.0, None, mybir.AluOpType.mult
        )
        nc.vector.tensor_copy(pack3[0:1, 1:2], rw[:])
        gws = singles.tile([1, 1], F32)
        nc.vector.tensor_tensor(gws[:], gamma[:], wscale[:], op=mybir.AluOpType.mult)
        nc.vector.tensor_scalar(
            pack3[0:1, 2:3], gws[:], 2.0 ** -7, None, mybir.AluOpType.mult
        )
        bp3 = psum_pool.tile([128, 3], F32, tag="ps", name="bp3")
        nc.tensor.matmul(bp3[:], ones_row[:], pack3[:], start=True, stop=True)
        b3 = singles.tile([128, 3], F32)
        nc.vector.tensor_copy(b3[:], bp3[:])
        s_x = b3[:, 0:1]
        r_w = b3[:, 1:2]
        s_o = b3[:, 2:3]

        # ---- bias, transposed: bias_t[p, c] = bias[c*128 + p] ----
        bias_t = singles.tile([128, OUT_F // 128], F32)
        nc.gpsimd.dma_start(bias_t[:], bias[:].rearrange("(c p) -> p c", p=128))

        # ---- group-0 W quantize emitted BEFORE the x quantize so its DVE
        # work runs during the x AllGather window (w_scale arrives earlier).
        # win DMAs ride the sync ring, which drains x-stats first.
        def emit_w_k(g, wqp, wqe, k):
            of0 = g * 512
            win = win_pool.tile([128, 512], F32, tag="win", name=f"win{g}_{k}")
            nc.sync.dma_start(
                win[:], wT[k * 128 : (k + 1) * 128, of0 : of0 + 512]
            )
            nc.scalar.activation(
                win[:], win[:], mybir.ActivationFunctionType.Copy, scale=r_w
            )
            nc.vector.tensor_scalar(
                win[:], win[:], 1.0, -1.0, mybir.AluOpType.min,
                mybir.AluOpType.max,
            )
            dst = wqp[k // 2][:, k % 2, :] if k < FK else wqe[k][:]
            nc.vector.tensor_scalar(
                dst, win[:], MAGIC, MAGIC, mybir.AluOpType.add,
                mybir.AluOpType.subtract,
            )

        def alloc_wq(g):
            wqp, wqe = {}, {}
            for kp in range(FK // 2):
                wqp[kp] = wq_pool.tile([128, 2, 512], FP8, tag="wq", name=f"wqp{g}_{kp}")
            for k in range(FK, KT):
                wqe[k] = wq_pool.tile([128, 512], BF16, tag="wq", name=f"wqe{g}_{k}")
            return wqp, wqe

        def emit_w_group(g):
            wqp, wqe = alloc_wq(g)
            for k in range(KT):
                emit_w_k(g, wqp, wqe, k)
            return wqp, wqe

        # ---- x quantize, interleaved with group-0's W quantize so the DVE
        # FIFO produces each matmul's two operands adjacently (all-W-then-
        # all-x left the exact x tiles ~20us behind the PE's first need)
        xq8 = {}   # paired: [128, 2, TPC] fp8 = (e4m3(x_q[k0]), e4m3(x_q[k1]))
        xqe = {}   # exact:  [128, TPC] bf16 x_q

        def emit_x_k(k):
            if k < FK:
                xt = xres[k]
            else:
                xt = xin_pool.tile([128, TPC], F32, tag="xin", name=f"xin{k}")
                reng = nc.gpsimd if k % 2 == 0 else nc.scalar
                reng.dma_start(xt[:], xT[k * 128 : (k + 1) * 128, :])
            nc.scalar.activation(
                xt[:], xt[:], mybir.ActivationFunctionType.Copy, scale=s_x,
                bias=MAGIC,
            )
            if k < FK:
                if k % 2 == 0:
                    xq8[k // 2] = xq_pool.tile(
                        [128, 2, TPC], FP8, tag="xq", name=f"xq8_{k // 2}"
                    )
                dst = xq8[k // 2][:, k % 2, :]
            else:
                xqe[k] = xq_pool.tile([128, TPC], BF16, tag="xq", name=f"xqe_{k}")
                dst = xqe[k][:]
            nc.vector.tensor_scalar(
                dst, xt[:], MAGIC, 127.0,
                mybir.AluOpType.subtract, mybir.AluOpType.min,
            )

        wq_g = alloc_wq(0)
        for k in range(KT):
            emit_w_k(0, wq_g[0], wq_g[1], k)
            emit_x_k(k)
        sctx.close()  # release resident x + stats SBUF
        ost_pool = ep(tc.tile_pool(name="ost", bufs=4))

        # ---- main loop ----
        def emit_mms(g, wqp, wqe, c, h, which):
            ps_name = f"ps{g * CPG + c}_{h}"
            ps = psum_tiles[(c, h)]
            cs = slice(c * 128, (c + 1) * 128)
            hs = slice(h * 512, (h + 1) * 512)
            n_mm = FK // 2 + (KT - FK)
            if which in ("paired", "all"):
                for kp in range(FK // 2):
                    nc.tensor.matmul(
                        ps[:], wqp[kp][:, :, cs], xq8[kp][:, :, hs],
                        start=(kp == 0), stop=False, perf_mode=DR,
                    )
            if which in ("exact", "all"):
                for k in range(FK, KT):
                    nc.tensor.matmul(
                        ps[:], wqe[k][:, cs], xqe[k][:, hs],
                        start=False, stop=(k == KT - 1),
                    )

        for g in range(NG):
            wqp, wqe = wq_g
            psum_tiles = {
                (c, h): psum_pool.tile(
                    [128, 512], F32, tag="ps", name=f"ps{g * CPG + c}_{h}"
                )
                for c in range(CPG)
                for h in range(2)
            }
            if g == 0:
                # resident-k matmuls for every chunk first: PE stays fed
                # while the exact-tile re-read streams in
                for c in range(CPG):
                    for h in range(2):
                        emit_mms(g, wqp, wqe, c, h, "paired")
                for c in range(CPG):
                    for h in range(2):
                        emit_mms(g, wqp, wqe, c, h, "exact")
            else:
                for c in range(CPG):
                    for h in range(2):
                        emit_mms(g, wqp, wqe, c, h, "all")
            if g + 1 < NG:
                wq_g = emit_w_group(g + 1)
            for c in range(CPG):
                chunk = g * CPG + c
                for h in range(2):
                    osb = ost_pool.tile(
                        [128, 512], F32, tag="ost", name=f"osb{chunk}_{h}"
                    )
                    # out = psum * s_o + bias[of]; single ACT-engine op
                    nc.scalar.activation(
                        osb[:], psum_tiles[(c, h)][:],
                        mybir.ActivationFunctionType.Identity,
                        scale=s_o, bias=bias_t[:, chunk : chunk + 1],
                    )
                    nc.scalar.dma_start(
                        outT[chunk * 128 : (chunk + 1) * 128,
                             h * 512 : (h + 1) * 512],
                        osb[:],
                    )

    nc.compile()
    return nc


def _prep_inputs(x, weight, bias):
    x2 = np.ascontiguousarray(x.reshape(TOKENS, IN_F).T)  # [IN_F, TOKENS]
    wT = np.ascontiguousarray(weight.T)  # [IN_F, OUT_F]
    in_maps = []
    for i in range(N_CORES):
        in_maps.append(
            {
                "xT": np.ascontiguousarray(x2[:, i * TPC : (i + 1) * TPC]),
                "wT": wT,
                "wS": np.ascontiguousarray(wT[:, i * OSL : (i + 1) * OSL]),
                "bias": bias,
            }
        )
    return in_maps


def _run(x, weight, bias, trace=False):
    if "nc" not in _cache:
        _cache["nc"] = _build()
    nc = _cache["nc"]
    in_maps = _prep_inputs(
        np.asarray(x, dtype=np.float32),
        np.asarray(weight, dtype=np.float32),
        np.asarray(bias, dtype=np.float32),
    )
    res = bass_utils.run_bass_kernel_spmd(
        nc, in_maps, list(range(N_CORES)), trace=trace
    )
    full = np.concatenate(
        [np.ascontiguousarray(res.results[i]["outT"].T) for i in range(N_CORES)],
        axis=0,
    )
    return full.reshape(4, 2048, OUT_F), res


def kernel(x, weight, bias):
    out, _ = _run(x, weight, bias)
    return out


# revision 27
# speedup vs baseline: 1.1211x; 1.1211x over previous
# BitLinear 1.58 (ternary-weight linear with int8 activation quantization)
# on 8 Trainium2 NeuronCores via Bass/Tile.
#
# Reference computation (fp32):
#   w_scale = max(mean(|W|), 1e-5)           (global over the full weight)
#   W_q     = clip(round(W / w_scale), -1, 1)          (ternary)
#   gamma   = max(max(|x|), 1e-5)            (global over the full activation)
#   x_q     = clip(round(x * 128/gamma), -128, 127)
#   out     = (x_q @ W_q^T) * (gamma*w_scale/128) + bias
#
# Sharding: data-parallel over the 8192 tokens (1024 tokens/core), weight
# replicated. Global scales via per-core partial stats + tiny AllGathers
# (w-stats gathered first so the W pipeline warms while x-stats finish).
#
# Matmul: W_q is the STATIONARY operand (of-chunks of 128 on PSUM
# partitions), x_q streams (512-token halves), output transposed
# ([OUT_F, TPC] per core, host transposes back). Of the 32 k-tiles:
#   - k < FK: fp8-e4m3 DoubleRow, two k-tiles per matmul (2 MACs/cell/cyc).
#     x_q rounds to e4m3; measured+simulated rel_err 0.0157 at FK=20 vs the
#     2e-2 gate (error scales ~sqrt(FK), 0.0196 at FK=32).
#   - k >= FK: exact bf16 matmuls (x_q in [-128,127] and W_q in {-1,0,1}
#     are exact in bf16; PSUM accumulates fp32, sums < 2^24).
# Mixing fp8-DoubleRow and bf16 matmuls in one PSUM accumulation group is
# fine - accumulation is fp32 either way.
#
# x is read once for stats; the FK paired k-tiles stay resident in SBUF
# (80 KB/part) so they quantize the moment gamma lands. The 12 exact
# k-tiles are re-read on the scalar ring (idle after the 8MB wS scan) and
# group 0's matmul emission is split resident-first so the PE has work for
# all 4 chunks while the re-read streams.
#
# Rounding: round-half-to-even (= jnp.round) done exactly in fp32 via the
# magic-constant trick (v + 1.5*2^23) - 1.5*2^23, fused into tensor_scalar.
# The int->e4m3 conversion on DVE output is RNE, matching the error sim.

import numpy as np
from contextlib import ExitStack

import concourse.bass as bass
import concourse.tile as tile
from concourse import bacc, mybir
from concourse import bass_utils

N_CORES = 8
IN_F = 4096
OUT_F = 4096
TOKENS = 8192  # 4 * 2048
TPC = TOKENS // N_CORES  # tokens per core = 1024
OSL = OUT_F // N_CORES  # per-core weight-stats slice = 512 out_features

KT = IN_F // 128  # 32 k-tiles
FK = 20  # k-tiles [0, FK) fp8-paired, [FK, KT) exact bf16
NG = OUT_F // 512  # 8 of-groups of 512
CPG = 4  # chunks (128 of) per group

MAGIC = 12582912.0  # 1.5 * 2**23: (v + MAGIC) - MAGIC == round-half-even(v)
EPS = 1e-5
F32 = mybir.dt.float32
FP8 = mybir.dt.float8e4
BF16 = mybir.dt.bfloat16
DR = mybir.MatmulPerfMode.DoubleRow

_cache = {}


def _build():
    nc = bacc.Bacc("TRN2", target_bir_lowering=False, debug=False, num_devices=N_CORES)
    xT = nc.dram_tensor("xT", [IN_F, TPC], F32, kind="ExternalInput").ap()
    wT = nc.dram_tensor("wT", [IN_F, OUT_F], F32, kind="ExternalInput").ap()
    wS = nc.dram_tensor("wS", [IN_F, OSL], F32, kind="ExternalInput").ap()
    bias = nc.dram_tensor("bias", [OUT_F], F32, kind="ExternalInput").ap()
    outT = nc.dram_tensor("outT", [OUT_F, TPC], F32, kind="ExternalOutput").ap()

    with tile.TileContext(nc) as tc, ExitStack() as ctx:
        ep = ctx.enter_context
        singles = ep(tc.tile_pool(name="singles", bufs=1))
        psum_pool = ep(tc.tile_pool(name="psum", bufs=8, space="PSUM"))
        dram = ep(tc.tile_pool(name="dram", bufs=1, space="DRAM"))
        # main pools first so the stats scope below is innermost (LIFO)
        xq8_pool = ep(tc.tile_pool(name="xq8", bufs=FK // 2))
        xqe_pool = ep(tc.tile_pool(name="xqe", bufs=(KT - FK) // 2))
        win_pool = ep(tc.tile_pool(name="win", bufs=3))
        wqp_pool = ep(tc.tile_pool(name="wqp", bufs=FK))
        wqe_pool = ep(tc.tile_pool(name="wqe", bufs=KT - FK))
        xin_pool = ep(tc.tile_pool(name="xin", bufs=2))
        # paired k-tiles of x stay resident through the stats phase
        sctx = ExitStack()
        xres_pool = sctx.enter_context(tc.tile_pool(name="xres", bufs=FK // 2))
        spw_pool = sctx.enter_context(tc.tile_pool(name="spw", bufs=2))

        ones_row = singles.tile([1, 128], F32)  # for partition-broadcast matmul
        nc.vector.memset(ones_row[:], 1.0)
        # warm-up AllGather: absorbs the collective launch/barrier cost
        # while stats stream
        warm_i = dram.tile([1], F32, tag="warm")
        warm_o = dram.tile([N_CORES], F32, tag="warmo")
        nc.scalar.dma_start(warm_i[:], ones_row[0:1, 0:1])
        nc.gpsimd.collective_compute(
            "AllGather", mybir.AluOpType.bypass,
            replica_groups=[list(range(N_CORES))],
            ins=[warm_i.opt()], outs=[warm_o.opt()],
        )
        # ---- stats: x loads split across two rings (sync+gpsimd); k < FK
        # kept resident. wS streams on the scalar ring with |.| accumulated
        # by ACT so all three rings pull concurrently.
        NJ = KT // 2  # 16 double-k tiles, j covers k = (2j, 2j+1)
        RJ = FK // 2  # first 10 stay resident
        xm = singles.tile([128, NJ], F32)
        # partition p holds rows (j*256 + 2p, j*256 + 2p + 1) as one 8KB
        # contiguous line; W pair tiles use the IDENTICAL row mapping
        # (contraction is order-invariant when both operands agree)
        xv2 = xT[:].rearrange("(j p two) y -> j p (two y)", p=128, two=2)
        xres = []
        for j in range(NJ):
            if j < RJ:
                xt = xres_pool.tile([128, 2 * TPC], F32, tag="xres", name=f"xres{j}")
                xres.append(xt)
            else:
                xt = xin_pool.tile([128, 2 * TPC], F32, tag="xin", name=f"spx{j}")
            eng = nc.sync if j % 2 == 0 else nc.gpsimd
            eng.dma_start(xt[:], xv2[j])
            nc.vector.tensor_reduce(
                xm[:, j : j + 1], xt[:], axis=mybir.AxisListType.X,
                op=mybir.AluOpType.max, apply_absolute_value=True,
            )
        SW = 1024
        wrows = SW // OSL
        NWS = IN_F // (128 * wrows)
        wv = wS[:].rearrange("(a p x) y -> a p (x y)", p=128, x=wrows)
        wm = singles.tile([128, NWS], F32)
        for j in range(NWS):
            st = spw_pool.tile([128, SW], F32, tag="spw", name=f"sw{j}")
            nc.scalar.dma_start(st[:], wv[j])
            nc.scalar.activation(
                st[:], st[:], mybir.ActivationFunctionType.Abs,
                accum_out=wm[:, j : j + 1],
            )

        # fold [128,N] -> [1,1] (cross-partition via DMA reshape). The tiny
        # DMAs ride the ring whose big reads gate them anyway (w-side ->
        # scalar behind wS, x-side -> sync behind x-stats), so they never
        # queue behind unrelated megabytes.
        def fold(src, op, nm, eng):
            c = singles.tile([128, 1], F32, tag=f"{nm}c")
            nc.vector.tensor_reduce(c[:], src[:], axis=mybir.AxisListType.X, op=op)
            t = singles.tile([1, 128], F32, tag=f"{nm}t")
            eng.dma_start(t[:], c[:])
            r = singles.tile([1, 1], F32, tag=f"{nm}r")
            nc.vector.tensor_reduce(r[:], t[:], axis=mybir.AxisListType.X, op=op)
            return r

        wsum = fold(wm, mybir.AluOpType.add, "ws", nc.scalar)
        gx = fold(xm, mybir.AluOpType.max, "gx", nc.sync)


        def newton_recip(name, src):
            # correctly-rounded-ish 1/src: HW reciprocal + one Newton step
            r0 = singles.tile([1, 1], F32, tag=f"{name}r0")
            nc.vector.reciprocal(r0[:], src[:])
            t = singles.tile([1, 1], F32, tag=f"{name}t")
            nc.vector.tensor_tensor(t[:], src[:], r0[:], op=mybir.AluOpType.mult)
            u = singles.tile([1, 1], F32, tag=f"{name}u")
            nc.vector.tensor_scalar(
                u[:], t[:], -1.0, 2.0, mybir.AluOpType.mult, mybir.AluOpType.add
            )
            r1 = singles.tile([1, 1], F32, tag=f"{name}r1")
            nc.vector.tensor_tensor(r1[:], r0[:], u[:], op=mybir.AluOpType.mult)
            return r1

        # ---- one combined AllGather for both stats ----
        cc_sb = singles.tile([1, 2], F32)
        nc.vector.tensor_copy(cc_sb[0:1, 0:1], gx[:])
        nc.vector.tensor_copy(cc_sb[0:1, 1:2], wsum[:])
        cin = dram.tile([2], F32, tag="cci")
        cout = dram.tile([2 * N_CORES], F32, tag="cco")
        nc.sync.dma_start(cin[:], cc_sb[:])
        nc.gpsimd.collective_compute(
            "AllGather", mybir.AluOpType.bypass,
            replica_groups=[list(range(N_CORES))],
            ins=[cin.opt()], outs=[cout.opt()],
        )
        g16 = singles.tile([1, 2 * N_CORES], F32)
        nc.sync.dma_start(g16[:], cout[:])
        g3 = g16[:].rearrange("p (r two) -> p two r", two=2)

        gsum = singles.tile([1, 1], F32)
        nc.vector.tensor_reduce(
            gsum[:], g3[0:1, 1:2, :], axis=mybir.AxisListType.X,
            op=mybir.AluOpType.add,
        )
        wscale = singles.tile([1, 1], F32)
        nc.vector.tensor_scalar(
            wscale[:], gsum[:], 1.0 / (OUT_F * IN_F), EPS,
            mybir.AluOpType.mult, mybir.AluOpType.max,
        )
        gmax = singles.tile([1, 1], F32)
        nc.vector.tensor_reduce(
            gmax[:], g3[0:1, 0:1, :], axis=mybir.AxisListType.X,
            op=mybir.AluOpType.max,
        )
        gamma = singles.tile([1, 1], F32)
        nc.vector.tensor_scalar(gamma[:], gmax[:], EPS, None, mybir.AluOpType.max)
        rw = newton_recip("rw", wscale)  # 1/w_scale
        rg = newton_recip("rg", gamma)   # 1/gamma
        pack3 = singles.tile([1, 3], F32)
        nc.vector.tensor_scalar(
            pack3[0:1, 0:1], rg[:], 128.0, None, mybir.AluOpType.mult
        )
        nc.vector.tensor_copy(pack3[0:1, 1:2], rw[:])
        gws = singles.tile([1, 1], F32)
        nc.vector.tensor_tensor(gws[:], gamma[:], wscale[:], op=mybir.AluOpType.mult)
        nc.vector.tensor_scalar(
            pack3[0:1, 2:3], gws[:], 2.0 ** -7, None, mybir.AluOpType.mult
        )
        bp3 = psum_pool.tile([128, 3], F32, tag="ps", name="bp3")
        nc.tensor.matmul(bp3[:], ones_row[:], pack3[:], start=True, stop=True)
        b3 = singles.tile([128, 3], F32)
        nc.vector.tensor_copy(b3[:], bp3[:])
        s_x = b3[:, 0:1]
        r_w = b3[:, 1:2]
        s_o = b3[:, 2:3]

        # ---- bias, transposed: bias_t[p, c] = bias[c*128 + p] ----
        bias_t = singles.tile([128, OUT_F // 128], F32)
        nc.gpsimd.dma_start(bias_t[:], bias[:].rearrange("(c p) -> p c", p=128))

        # ---- group-0 W quantize emitted BEFORE the x quantize so its DVE
        # work runs during the x AllGather window (w_scale arrives earlier).
        # win DMAs ride the sync ring, which drains x-stats first.
        def emit_w_k(g, wqp, wqe, j):
            of0 = g * 512
            win = win_pool.tile([128, 2, 512], F32, tag="win", name=f"win{g}_{j}")
            nc.sync.dma_start(
                win[:],
                wT[j * 256 : (j + 1) * 256, of0 : of0 + 512].rearrange(
                    "(p two) y -> p two y", two=2
                ),
            )
            nc.scalar.activation(
                win[:], win[:], mybir.ActivationFunctionType.Copy, scale=r_w
            )
            nc.vector.tensor_scalar(
                win[:], win[:], 1.0, -1.0, mybir.AluOpType.min,
                mybir.AluOpType.max,
            )
            dst = wqp[j] if j < RJ else wqe[j]
            nc.vector.tensor_scalar(
                dst[:], win[:], MAGIC, MAGIC, mybir.AluOpType.add,
                mybir.AluOpType.subtract,
            )

        def alloc_wq(g):
            wqp, wqe = {}, {}
            for j in range(RJ):
                wqp[j] = wqp_pool.tile([128, 2, 512], FP8, tag="wqp", name=f"wqp{g}_{j}")
            for j in range(RJ, NJ):
                wqe[j] = wqe_pool.tile([128, 2, 512], BF16, tag="wqe", name=f"wqe{g}_{j}")
            return wqp, wqe

        def emit_w_group(g):
            wqp, wqe = alloc_wq(g)
            for j in range(NJ):
                emit_w_k(g, wqp, wqe, j)
            return wqp, wqe

        # ---- x quantize, interleaved with group-0's W quantize so the DVE
        # FIFO produces each matmul's two operands adjacently (all-W-then-
        # all-x left the exact x tiles ~20us behind the PE's first need)
        xq8 = {}   # paired: [128, 2, TPC] fp8 = (e4m3(x_q[k0]), e4m3(x_q[k1]))
        xqe = {}   # exact:  [128, TPC] bf16 x_q

        def emit_x_k(j):
            if j < RJ:
                xt = xres[j]
            else:
                xt = xin_pool.tile([128, 2 * TPC], F32, tag="xin", name=f"xin{j}")
                reng = nc.gpsimd if j % 2 == 0 else nc.scalar
                reng.dma_start(xt[:], xv2[j])
            nc.scalar.activation(
                xt[:], xt[:], mybir.ActivationFunctionType.Copy, scale=s_x,
                bias=MAGIC,
            )
            if j < RJ:
                xq8[j] = xq8_pool.tile(
                    [128, 2, TPC], FP8, tag="xq8", name=f"xq8_{j}"
                )
                dst = xq8[j]
            else:
                xqe[j] = xqe_pool.tile(
                    [128, 2, TPC], BF16, tag="xqe", name=f"xqe_{j}"
                )
                dst = xqe[j]
            nc.vector.tensor_scalar(
                dst[:], xt[:].rearrange("p (two y) -> p two y", two=2),
                MAGIC, 127.0,
                mybir.AluOpType.subtract, mybir.AluOpType.min,
            )

        wq_g = alloc_wq(0)
        for j in range(NJ):
            emit_w_k(0, wq_g[0], wq_g[1], j)
            emit_x_k(j)
        sctx.close()  # release resident x + stats SBUF
        ost_pool = ep(tc.tile_pool(name="ost", bufs=4))

        # ---- main loop ----
        def emit_mms(g, wqp, wqe, c, h, which):
            ps = psum_tiles[(c, h)]
            cs = slice(c * 128, (c + 1) * 128)
            hs = slice(h * 512, (h + 1) * 512)
            if which in ("paired", "all"):
                for j in range(RJ):
                    nc.tensor.matmul(
                        ps[:], wqp[j][:, :, cs], xq8[j][:, :, hs],
                        start=(j == 0), stop=False, perf_mode=DR,
                    )
            if which in ("exact", "all"):
                for j in range(RJ, NJ):
                    for half in range(2):
                        nc.tensor.matmul(
                            ps[:], wqe[j][:, half, cs],
                            xqe[j][:, half, hs],
                            start=False,
                            stop=(j == NJ - 1 and half == 1),
                        )

        for g in range(NG):
            wqp, wqe = wq_g
            psum_tiles = {
                (c, h): psum_pool.tile(
                    [128, 512], F32, tag="ps", name=f"ps{g * CPG + c}_{h}"
                )
                for c in range(CPG)
                for h in range(2)
            }
            if g == 0:
                # resident-k matmuls for every chunk first: PE stays fed
                # while the exact-tile re-read streams in
                for c in range(CPG):
                    for h in range(2):
                        emit_mms(g, wqp, wqe, c, h, "paired")
                for c in range(CPG):
                    for h in range(2):
                        emit_mms(g, wqp, wqe, c, h, "exact")
            else:
                for c in range(CPG):
                    for h in range(2):
                        emit_mms(g, wqp, wqe, c, h, "all")
            if g + 1 < NG:
                wq_g = emit_w_group(g + 1)
            for c in range(CPG):
                chunk = g * CPG + c
                for h in range(2):
                    osb = ost_pool.tile(
                        [128, 512], F32, tag="ost", name=f"osb{chunk}_{h}"
                    )
                    # out = psum * s_o + bias[of]; single ACT-engine op
                    nc.scalar.activation(
                        osb[:], psum_tiles[(c, h)][:],
                        mybir.ActivationFunctionType.Identity,
                        scale=s_o, bias=bias_t[:, chunk : chunk + 1],
                    )
                    nc.scalar.dma_start(
                        outT[chunk * 128 : (chunk + 1) * 128,
                             h * 512 : (h + 1) * 512],
                        osb[:],
                    )

    nc.compile()
    return nc


def _prep_inputs(x, weight, bias):
    x2 = np.ascontiguousarray(x.reshape(TOKENS, IN_F).T)  # [IN_F, TOKENS]
    wT = np.ascontiguousarray(weight.T)  # [IN_F, OUT_F]
    in_maps = []
    for i in range(N_CORES):
        in_maps.append(
            {
                "xT": np.ascontiguousarray(x2[:, i * TPC : (i + 1) * TPC]),
                "wT": wT,
                "wS": np.ascontiguousarray(wT[:, i * OSL : (i + 1) * OSL]),
                "bias": bias,
            }
        )
    return in_maps


def _run(x, weight, bias, trace=False):
    if "nc" not in _cache:
        _cache["nc"] = _build()
    nc = _cache["nc"]
    in_maps = _prep_inputs(
        np.asarray(x, dtype=np.float32),
        np.asarray(weight, dtype=np.float32),
        np.asarray(bias, dtype=np.float32),
    )
    res = bass_utils.run_bass_kernel_spmd(
        nc, in_maps, list(range(N_CORES)), trace=trace
    )
    full = np.concatenate(
        [np.ascontiguousarray(res.results[i]["outT"].T) for i in range(N_CORES)],
        axis=0,
    )
    return full.reshape(4, 2048, OUT_F), res


def kernel(x, weight, bias):
    out, _ = _run(x, weight, bias)
    return out
